# revision 7
# baseline (speedup 1.0000x reference)
"""Sparse-attention SPMD kernel (one NeuronCore program), v2.

Per core: B=2 batches x NH=8 heads as 4*B head-pairs (A, B):
  - QK projections fp32, col-paired A/B (col_grp concurrency), PSUM->SBUF
    copies alternate ACT/DVE
  - V projection batched across all 8 heads per (batch, m-tile):
    stationary hTb tile, rhs = packed W_V [128, NH*D] bf16, N=512
  - S = Q^T.T @ K^T fp32 K=64, 4-way array-tiled: row groups A/B x col
    halves of the q-tile run concurrently
  - mask + negmax fused: DVE tensor_tensor_reduce computes
    ms = -(S + maskbias) into SBUF and accum-min = -(masked rowmax)
  - ACT: P = exp(-ms + negmax) = exp(S + mb - max), accum -> rowsum (bf16 P)
  - DVE: reciprocal; Pn = P * rcp (bf16 4x mode), written into 2-qt slabs
  - transpose: one DMA-xbar transpose per 2-qt slab [128, 2048] on the sync
    HWDGE ring only (other rings corrupt transposes)
  - AV: O^T[d,q] bf16 K=128 col-paired A/B; O copies PSUM->SBUF alternate
    ACT/DVE; output DMA on gpsimd ring
Output written as O^T [B, NH, 64, G]; host transposes to [B, NH, G, 64].
"""
import sys

sys.path.insert(0, '/opt/trn_rl_repo')
from contextlib import ExitStack

import concourse.bass as bass
import concourse.tile as tile
from concourse import bacc, mybir

FP32 = mybir.dt.float32
BF16 = mybir.dt.bfloat16
I32 = mybir.dt.int32
AF = mybir.ActivationFunctionType
ALU = mybir.AluOpType


def build_attention(B=2, NH=8, G=1024, I=256, D=64, tp_batch=2,
                    s_colsplit=True):
    """DRAM params: hT [B,I,G] f32, hTb [B,I,G] bf16, mbp [G,G] bf16
    (-1e30 where masked, 0 where allowed), wq/wk [NH,I,D] f32 (wq pre-scaled
    by 1/sqrt(D)), wvp [KT,128,NH*D] bf16 (packed); out [B,NH,D,G] f32 (O^T).
    """
    assert D == 64 and I % 128 == 0 and G % 512 == 0 and NH % 2 == 0
    KT = I // 128          # contraction k-tiles for projections
    QT = G // 128          # q tiles
    MC = G // 512          # m chunks of 512 (S rhs)
    QC = G // 512          # q chunks of 512 (AV psum)
    MT = G // 128          # m tiles
    Q2 = QT // 2           # 2-qt transpose slabs

    nc = bacc.Bacc(None, target_bir_lowering=False, debug=False)
    hT_ext = nc.declare_dram_parameter("hT", [B, I, G], FP32, isOutput=False)
    hTb_ext = nc.declare_dram_parameter("hTb", [B, I, G], BF16, isOutput=False)
    mbp_ext = nc.declare_dram_parameter("mbp", [G, G], BF16, isOutput=False)
    wq_ext = nc.declare_dram_parameter("wq", [NH, I, D], FP32, isOutput=False)
    wk_ext = nc.declare_dram_parameter("wk", [NH, I, D], FP32, isOutput=False)
    wvp_ext = nc.declare_dram_parameter("wvp", [KT, 128, NH * D], BF16,
                                        isOutput=False)
    id_ext = nc.declare_dram_parameter("ident", [128, 128], BF16,
                                       isOutput=False)
    out_ext = nc.declare_dram_parameter("out", [B, NH, D, G], FP32,
                                        isOutput=True)

    ctx = ExitStack()
    with ctx:
        tc = ctx.enter_context(tile.TileContext(nc))
        const = ctx.enter_context(tc.tile_pool(name="const", bufs=1))
        vpool = ctx.enter_context(tc.tile_pool(name="vsb", bufs=1))
        qk_pool = ctx.enter_context(tc.tile_pool(name="qk", bufs=2))
        p_pool = ctx.enter_context(tc.tile_pool(name="p", bufs=4))
        pn_pool = ctx.enter_context(tc.tile_pool(name="pn", bufs=3))
        pt_pool = ctx.enter_context(tc.tile_pool(name="pt", bufs=3))
        o_pool = ctx.enter_context(tc.tile_pool(name="o", bufs=2))
        st_pool = ctx.enter_context(tc.tile_pool(name="stats", bufs=2))
        # PSUM budget (8 banks): pss 3x2 + proj 1 + v/o shared 1 = 8
        ps_proj = ctx.enter_context(tc.tile_pool(name="psproj", bufs=1,
                                                 space="PSUM"))
        ps_s = ctx.enter_context(tc.tile_pool(name="pss", bufs=3,
                                              space="PSUM"))
        ps_vo = ctx.enter_context(tc.tile_pool(name="psvo", bufs=1,
                                               space="PSUM"))

        # ---------- setup: load inputs (gpsimd SWDGE ring; sync ring is
        # reserved for the xbar transposes) ----------
        hT_sb = const.tile([128, B, KT, G], FP32)
        hTb_sb = const.tile([128, B, KT, G], BF16)
        for b in range(B):
            nc.gpsimd.dma_start(
                out=hT_sb[:, b],
                in_=hT_ext[b].rearrange("(kt p) g -> p kt g", p=128))
            nc.gpsimd.dma_start(
                out=hTb_sb[:, b],
                in_=hTb_ext[b].rearrange("(kt p) g -> p kt g", p=128))

        wq_sb = const.tile([128, NH, KT, D], FP32)
        wk_sb = const.tile([128, NH, KT, D], FP32)
        wv_sb = const.tile([128, KT, NH * D], BF16)
        nc.gpsimd.dma_start(out=wq_sb[:],
                            in_=wq_ext.rearrange("h (kt p) d -> p h kt d", p=128))
        nc.gpsimd.dma_start(out=wk_sb[:],
                            in_=wk_ext.rearrange("h (kt p) d -> p h kt d", p=128))
        nc.gpsimd.dma_start(out=wv_sb[:],
                            in_=wvp_ext.rearrange("kt p hd -> p kt hd"))

        ident_sb = const.tile([128, 128], BF16)
        nc.gpsimd.dma_start(out=ident_sb[:], in_=id_ext[:])

        # mask bias, host-prepared bf16: [128, qt, m]
        mbp_sb = const.tile([128, QT, G], BF16)
        nc.gpsimd.dma_start(
            out=mbp_sb[:],
            in_=mbp_ext.rearrange("(qt p) m -> p qt m", p=128))

        # V for both batches: [128(m), b, mt, NH*D] bf16
        v_sb = vpool.tile([128, B, MT, NH * D], BF16)

        def emit_v_batch(b):
            for mt in range(MT):
                psv = ps_vo.tile([128, 512], FP32, tag="vo", name=f"psv{b}{mt}")
                for kt in range(KT):
                    nc.tensor.matmul(
                        psv[:], hTb_sb[:, b, kt, 128 * mt:128 * (mt + 1)],
                        wv_sb[:, kt], start=(kt == 0), stop=(kt == KT - 1))
                if mt % 2 == 0:
                    nc.scalar.copy(v_sb[:, b, mt], psv[:])
                else:
                    nc.vector.tensor_copy(v_sb[:, b, mt], psv[:])

        # ---------- main loop over (batch, head-pair), software-pipelined ----
        pairs = [(b, hp) for b in range(B) for hp in range(NH // 2)]
        rows_of = {"A": slice(0, 64), "B": slice(64, 128)}
        state = {}

        def emit_proj(i):
            b, hp = pairs[i]
            hA, hB = 2 * hp, 2 * hp + 1
            qk_sb = qk_pool.tile([128, 2 * G], FP32, tag="qk", name=f"qk{i}")
            ncopy = 0
            for W, w_sb, off in (("q", wq_sb, 0), ("k", wk_sb, G)):
                for chk in range(G // 512):
                    sl = slice(512 * chk, 512 * (chk + 1))
                    psp = ps_proj.tile([128, 512], FP32, tag="proj",
                                       name=f"psp{i}{W}{chk}")
                    for kt in range(KT):
                        st, sp = kt == 0, kt == KT - 1
                        nc.tensor.matmul(psp[0:64, :], w_sb[:, hA, kt],
                                         hT_sb[:, b, kt, sl], start=st, stop=sp,
                                         skip_group_check=True)
                        nc.tensor.matmul(psp[64:128, :], w_sb[:, hB, kt],
                                         hT_sb[:, b, kt, sl], start=st, stop=sp,
                                         skip_group_check=True)
                    dst = qk_sb[:, off + 512 * chk:off + 512 * (chk + 1)]
                    if ncopy % 2 == 0:
                        nc.scalar.copy(dst, psp[:])
                    else:
                        nc.vector.tensor_copy(dst, psp[:])
                    ncopy += 1
            state[i] = qk_sb

        emit_v_batch(0)
        emit_proj(0)
        for i, (b, hp) in enumerate(pairs):
            hA, hB = 2 * hp, 2 * hp + 1
            qk_sb = state.pop(i)

            negmax = {X: st_pool.tile([128, QT], FP32, tag=f"negmax{X}",
                                  name=f"negmax{X}{i}") for X in "AB"}
            rowsum = {X: st_pool.tile([128, QT], FP32, tag=f"rowsum{X}",
                                      name=f"rowsum{X}{i}") for X in "AB"}
            rcp = {X: st_pool.tile([128, QT], FP32, tag=f"rcp{X}",
                                   name=f"rcp{X}{i}") for X in "AB"}
            # PT layout: [m_in(128), qt2, qt_lo(2), mt(8), q_in(128)]
            PT = {X: pt_pool.tile([128, Q2, 2, MT, 128], BF16, tag="pt",
                                  name=f"PT{X}{i}")
                  for X in "AB"}
            pn2 = {}
            for qt in range(QT):
                pss = {X: ps_s.tile([128, G], FP32, tag="s",
                                    name=f"pss{X}{i}{qt}") for X in "AB"}
                # S: 4-way tiled (A/B row groups x q col halves), group
                # stays open; the identity-matmul mask add closes it.
                for chk in range(MC):
                    sl = slice(512 * chk, 512 * (chk + 1))
                    for X in "AB":
                        rows = rows_of[X]
                        rbase = rows.start
                        if s_colsplit:
                            for c in range(2):
                                qsl = slice(128 * qt + 64 * c,
                                            128 * qt + 64 * (c + 1))
                                nc.tensor.matmul(
                                    pss[X][64 * c:64 * (c + 1), sl],
                                    qk_sb[rows, qsl],
                                    qk_sb[rows, G + 512 * chk:G + 512 * (chk + 1)],
                                    start=True, stop=False,
                                    tile_position=(rbase, 64 * c),
                                    skip_group_check=True)
                        else:
                            nc.tensor.matmul(
                                pss[X][:, sl],
                                qk_sb[rows, 128 * qt:128 * (qt + 1)],
                                qk_sb[rows, G + 512 * chk:G + 512 * (chk + 1)],
                                start=True, stop=False,
                                skip_group_check=True)
                # mask add via identity matmul (PE), both heads
                for X in "AB":
                    for chk in range(MC):
                        sl = slice(512 * chk, 512 * (chk + 1))
                        nc.tensor.matmul(
                            pss[X][:, sl], ident_sb[:], mbp_sb[:, qt, sl],
                            start=False, stop=(chk == MC - 1),
                            skip_group_check=True)
                for X in "AB":
                    nc.vector.tensor_reduce(
                        negmax[X][:, qt:qt + 1], pss[X][:],
                        axis=mybir.AxisListType.X, op=ALU.max, negate=True)
                    # P = exp(S + mb - max), rowsum
                    P = p_pool.tile([128, G], BF16, tag="p", name=f"P{X}{i}{qt}")
                    nc.scalar.activation(
                        P[:], pss[X][:], AF.Exp,
                        bias=negmax[X][:, qt:qt + 1], scale=1.0,
                        accum_out=rowsum[X][:, qt:qt + 1])
                    nc.vector.reciprocal(rcp[X][:, qt:qt + 1],
                                         rowsum[X][:, qt:qt + 1])
                    if qt % tp_batch == 0:
                        pn2[X] = pn_pool.tile([128, tp_batch, G], BF16,
                                              tag="pn", name=f"pn{X}{i}{qt}")
                    nc.vector.tensor_scalar_mul(pn2[X][:, qt % tp_batch], P[:],
                                                rcp[X][:, qt:qt + 1])
                    if qt % tp_batch == tp_batch - 1:
                        if tp_batch == 2:
                            tp_out = PT[X][:, qt // 2]
                        else:
                            tp_out = PT[X][:, qt // 2, qt % 2]
                        nc.sync.dma_start_transpose(out=tp_out, in_=pn2[X][:])

            # next pair's projections fill the PE bubble while softmax drains
            if i + 1 < len(pairs):
                if hp == NH // 2 - 1:
                    emit_v_batch(b + 1)
                emit_proj(i + 1)

            # --- AV (bf16 K=128, col-paired) + output ---
            o_sb = o_pool.tile([128, QC, 512], FP32, tag="o", name=f"o{i}")
            for qc in range(QC):
                pso = ps_vo.tile([128, 512], FP32, tag="vo", name=f"pso{i}{qc}")
                for mkt in range(MT):
                    st, sp = mkt == 0, mkt == MT - 1
                    nc.tensor.matmul(
                        pso[0:64, :],
                        v_sb[:, b, mkt, 64 * hA:64 * (hA + 1)],
                        PT["A"][:, 2 * qc:2 * qc + 2, :, mkt, :],
                        start=st, stop=sp, skip_group_check=True)
                    nc.tensor.matmul(
                        pso[64:128, :],
                        v_sb[:, b, mkt, 64 * hB:64 * (hB + 1)],
                        PT["B"][:, 2 * qc:2 * qc + 2, :, mkt, :],
                        start=st, stop=sp, skip_group_check=True)
                if qc % 2 == 0:
                    nc.scalar.copy(o_sb[:, qc], pso[:])
                else:
                    nc.vector.tensor_copy(o_sb[:, qc], pso[:])
            nc.gpsimd.dma_start(
                out=out_ext[b, hA].rearrange("d (qc qi) -> d qc qi", qc=QC),
                in_=o_sb[0:64])
            nc.gpsimd.dma_start(
                out=out_ext[b, hB].rearrange("d (qc qi) -> d qc qi", qc=QC),
                in_=o_sb[64:128])

    nc.compile()
    return nc


# ---------------------------------------------------------------------------
# Host-side wrapper: shard over batch across 8 cores, run SPMD, gather.
# ---------------------------------------------------------------------------
import numpy as np
import ml_dtypes

N_CORES = 8
_B_FULL, _NH, _G, _I, _D = 16, 8, 1024, 256, 64
_B_PER_CORE = _B_FULL // N_CORES
_KT = _I // 128

_cached_nc = None


def _get_nc():
    global _cached_nc
    if _cached_nc is None:
        _cached_nc = build_attention(B=_B_PER_CORE, NH=_NH, G=_G, I=_I, D=_D)
    return _cached_nc


def _make_in_maps(h, mask, W_Q, W_K, W_V):
    hT = np.ascontiguousarray(np.transpose(np.asarray(h, np.float32), (0, 2, 1)))
    hTb = hT.astype(ml_dtypes.bfloat16)
    wq = np.ascontiguousarray(np.asarray(W_Q, np.float32) / np.sqrt(np.float32(_D)))
    wk = np.ascontiguousarray(np.asarray(W_K, np.float32))
    # wvp: [KT, 128, NH*D] bf16 packed for the all-heads batched V projection
    wv = np.asarray(W_V, np.float32)                       # [NH, I, D]
    wvp = np.ascontiguousarray(
        wv.transpose(1, 0, 2).reshape(_KT, 128, _NH * _D)
    ).astype(ml_dtypes.bfloat16)
    # mask bias bf16: -1e30 where masked (mask==1), 0 where allowed
    mbp = np.where(np.asarray(mask) != 0, np.float32(-1e30),
                   np.float32(0.0)).astype(ml_dtypes.bfloat16)
    ident = np.eye(128).astype(ml_dtypes.bfloat16)
    return [
        {
            "hT": np.ascontiguousarray(hT[c * _B_PER_CORE:(c + 1) * _B_PER_CORE]),
            "hTb": np.ascontiguousarray(hTb[c * _B_PER_CORE:(c + 1) * _B_PER_CORE]),
            "mbp": mbp,
            "wq": wq,
            "wk": wk,
            "wvp": wvp,
            "ident": ident,
        }
        for c in range(N_CORES)
    ]


def kernel(h, mask, W_Q, W_K, W_V):
    """h [16,1024,256] f32, mask [1024,1024] i32, W_* [8,256,64] f32
    -> [16, 8, 1024, 64] f32"""
    from concourse.bass_utils import run_bass_kernel_spmd

    nc = _get_nc()
    in_maps = _make_in_maps(h, mask, W_Q, W_K, W_V)
    res = run_bass_kernel_spmd(nc, in_maps, core_ids=list(range(N_CORES)))
    outs = [np.asarray(res.results[c]["out"]).reshape(_B_PER_CORE, _NH, _D, _G)
            for c in range(N_CORES)]
    full = np.concatenate(outs, axis=0)              # [16, NH, D, G]
    return np.ascontiguousarray(full.transpose(0, 1, 3, 2)).astype(np.float32)


# revision 8
# speedup vs baseline: 1.1104x; 1.1104x over previous
"""Sparse-attention SPMD kernel (one NeuronCore program), v2.

Per core: B=2 batches x NH=8 heads as 4*B head-pairs (A, B):
  - QK projections fp32, col-paired A/B (col_grp concurrency), PSUM->SBUF
    copies alternate ACT/DVE
  - V projection batched across all 8 heads per (batch, m-tile):
    stationary hTb tile, rhs = packed W_V [128, NH*D] bf16, N=512
  - S = Q^T.T @ K^T fp32 K=64, 4-way array-tiled: row groups A/B x col
    halves of the q-tile run concurrently
  - mask + negmax fused: DVE tensor_tensor_reduce computes
    ms = -(S + maskbias) into SBUF and accum-min = -(masked rowmax)
  - ACT: P = exp(-ms + negmax) = exp(S + mb - max), accum -> rowsum (bf16 P)
  - DVE: reciprocal; Pn = P * rcp (bf16 4x mode), written into 2-qt slabs
  - transpose: one DMA-xbar transpose per 2-qt slab [128, 2048] on the sync
    HWDGE ring only (other rings corrupt transposes)
  - AV: O^T[d,q] bf16 K=128 col-paired A/B; O copies PSUM->SBUF alternate
    ACT/DVE; output DMA on gpsimd ring
Output written as O^T [B, NH, 64, G]; host transposes to [B, NH, G, 64].
"""
import sys

sys.path.insert(0, '/opt/trn_rl_repo')
from contextlib import ExitStack

import concourse.bass as bass
import concourse.tile as tile
from concourse import bacc, mybir

FP32 = mybir.dt.float32
BF16 = mybir.dt.bfloat16
I32 = mybir.dt.int32
AF = mybir.ActivationFunctionType
ALU = mybir.AluOpType


def build_attention(B=2, NH=8, G=1024, I=256, D=64, tp_batch=2,
                    s_colsplit=False):
    """DRAM params: hT [B,I,G] f32, hTb [B,I,G] bf16, mbp [G,G] bf16
    (-1e30 where masked, 0 where allowed), wq/wk [NH,I,D] f32 (wq pre-scaled
    by 1/sqrt(D)), wvp [KT,128,NH*D] bf16 (packed); out [B,NH,D,G] f32 (O^T).
    """
    assert D == 64 and I % 128 == 0 and G % 512 == 0 and NH % 2 == 0
    KT = I // 128          # contraction k-tiles for projections
    QT = G // 128          # q tiles
    MC = G // 512          # m chunks of 512 (S rhs)
    QC = G // 512          # q chunks of 512 (AV psum)
    MT = G // 128          # m tiles
    Q2 = QT // 2           # 2-qt transpose slabs

    nc = bacc.Bacc(None, target_bir_lowering=False, debug=False)
    hT_ext = nc.declare_dram_parameter("hT", [B, I, G], FP32, isOutput=False)
    hTb_ext = nc.declare_dram_parameter("hTb", [B, I, G], BF16, isOutput=False)
    mbp_ext = nc.declare_dram_parameter("mbp", [G, G], BF16, isOutput=False)
    wq_ext = nc.declare_dram_parameter("wq", [NH, I, D], FP32, isOutput=False)
    wk_ext = nc.declare_dram_parameter("wk", [NH, I, D], FP32, isOutput=False)
    wvp_ext = nc.declare_dram_parameter("wvp", [KT, 128, NH * D], BF16,
                                        isOutput=False)
    id_ext = nc.declare_dram_parameter("ident", [128, 128], BF16,
                                       isOutput=False)
    out_ext = nc.declare_dram_parameter("out", [B, NH, D, G], FP32,
                                        isOutput=True)

    ctx = ExitStack()
    with ctx:
        tc = ctx.enter_context(tile.TileContext(nc))
        const = ctx.enter_context(tc.tile_pool(name="const", bufs=1))
        vpool = ctx.enter_context(tc.tile_pool(name="vsb", bufs=1))
        qk_pool = ctx.enter_context(tc.tile_pool(name="qk", bufs=2))
        p_pool = ctx.enter_context(tc.tile_pool(name="p", bufs=4))
        pn_pool = ctx.enter_context(tc.tile_pool(name="pn", bufs=3))
        pt_pool = ctx.enter_context(tc.tile_pool(name="pt", bufs=3))
        o_pool = ctx.enter_context(tc.tile_pool(name="o", bufs=2))
        st_pool = ctx.enter_context(tc.tile_pool(name="stats", bufs=2))
        # PSUM budget (8 banks): pss 3x2 + proj 1 + v/o shared 1 = 8
        ps_proj = ctx.enter_context(tc.tile_pool(name="psproj", bufs=1,
                                                 space="PSUM"))
        ps_s = ctx.enter_context(tc.tile_pool(name="pss", bufs=3,
                                              space="PSUM"))
        ps_vo = ctx.enter_context(tc.tile_pool(name="psvo", bufs=1,
                                               space="PSUM"))

        # ---------- setup: load inputs (gpsimd SWDGE ring; sync ring is
        # reserved for the xbar transposes) ----------
        hT_sb = const.tile([128, B, KT, G], FP32)
        hTb_sb = const.tile([128, B, KT, G], BF16)
        for b in range(B):
            nc.gpsimd.dma_start(
                out=hT_sb[:, b],
                in_=hT_ext[b].rearrange("(kt p) g -> p kt g", p=128))
            nc.gpsimd.dma_start(
                out=hTb_sb[:, b],
                in_=hTb_ext[b].rearrange("(kt p) g -> p kt g", p=128))

        wq_sb = const.tile([128, NH, KT, D], FP32)
        wk_sb = const.tile([128, NH, KT, D], FP32)
        wv_sb = const.tile([128, KT, NH * D], BF16)
        nc.gpsimd.dma_start(out=wq_sb[:],
                            in_=wq_ext.rearrange("h (kt p) d -> p h kt d", p=128))
        nc.gpsimd.dma_start(out=wk_sb[:],
                            in_=wk_ext.rearrange("h (kt p) d -> p h kt d", p=128))
        nc.gpsimd.dma_start(out=wv_sb[:],
                            in_=wvp_ext.rearrange("kt p hd -> p kt hd"))

        ident_sb = const.tile([128, 128], BF16)
        nc.gpsimd.dma_start(out=ident_sb[:], in_=id_ext[:])

        # mask bias, host-prepared bf16: [128, qt, m]
        mbp_sb = const.tile([128, QT, G], BF16)
        nc.gpsimd.dma_start(
            out=mbp_sb[:],
            in_=mbp_ext.rearrange("(qt p) m -> p qt m", p=128))

        # V for both batches: [128(m), b, mt, NH*D] bf16
        v_sb = vpool.tile([128, B, MT, NH * D], BF16)

        def emit_v_batch(b):
            for mt in range(MT):
                psv = ps_vo.tile([128, 512], FP32, tag="vo", name=f"psv{b}{mt}")
                for kt in range(KT):
                    nc.tensor.matmul(
                        psv[:], hTb_sb[:, b, kt, 128 * mt:128 * (mt + 1)],
                        wv_sb[:, kt], start=(kt == 0), stop=(kt == KT - 1))
                if mt % 2 == 0:
                    nc.scalar.copy(v_sb[:, b, mt], psv[:])
                else:
                    nc.vector.tensor_copy(v_sb[:, b, mt], psv[:])

        # ---------- main loop over (batch, head-pair), software-pipelined ----
        pairs = [(b, hp) for b in range(B) for hp in range(NH // 2)]
        rows_of = {"A": slice(0, 64), "B": slice(64, 128)}
        state = {}

        def emit_proj(i):
            b, hp = pairs[i]
            hA, hB = 2 * hp, 2 * hp + 1
            qk_sb = qk_pool.tile([128, 2 * G], FP32, tag="qk", name=f"qk{i}")
            ncopy = 0
            for W, w_sb, off in (("q", wq_sb, 0), ("k", wk_sb, G)):
                for chk in range(G // 512):
                    sl = slice(512 * chk, 512 * (chk + 1))
                    psp = ps_proj.tile([128, 512], FP32, tag="proj",
                                       name=f"psp{i}{W}{chk}")
                    for kt in range(KT):
                        st, sp = kt == 0, kt == KT - 1
                        nc.tensor.matmul(psp[0:64, :], w_sb[:, hA, kt],
                                         hT_sb[:, b, kt, sl], start=st, stop=sp,
                                         skip_group_check=True)
                        nc.tensor.matmul(psp[64:128, :], w_sb[:, hB, kt],
                                         hT_sb[:, b, kt, sl], start=st, stop=sp,
                                         skip_group_check=True)
                    dst = qk_sb[:, off + 512 * chk:off + 512 * (chk + 1)]
                    if ncopy % 2 == 0:
                        nc.scalar.copy(dst, psp[:])
                    else:
                        nc.vector.tensor_copy(dst, psp[:])
                    ncopy += 1
            state[i] = qk_sb

        emit_v_batch(0)
        emit_proj(0)
        for i, (b, hp) in enumerate(pairs):
            hA, hB = 2 * hp, 2 * hp + 1
            qk_sb = state.pop(i)

            negmax = {X: st_pool.tile([128, QT], FP32, tag=f"negmax{X}",
                                  name=f"negmax{X}{i}") for X in "AB"}
            rowsum = {X: st_pool.tile([128, QT], FP32, tag=f"rowsum{X}",
                                      name=f"rowsum{X}{i}") for X in "AB"}
            rcp = {X: st_pool.tile([128, QT], FP32, tag=f"rcp{X}",
                                   name=f"rcp{X}{i}") for X in "AB"}
            # PT layout: [m_in(128), qt2, qt_lo(2), mt(8), q_in(128)]
            PT = {X: pt_pool.tile([128, Q2, 2, MT, 128], BF16, tag="pt",
                                  name=f"PT{X}{i}")
                  for X in "AB"}
            pn2 = {}
            for qt in range(QT):
                pss = {X: ps_s.tile([128, G], FP32, tag="s",
                                    name=f"pss{X}{i}{qt}") for X in "AB"}
                # S: 4-way tiled (A/B row groups x q col halves), group
                # stays open; the identity-matmul mask add closes it.
                for chk in range(MC):
                    sl = slice(512 * chk, 512 * (chk + 1))
                    for X in "AB":
                        rows = rows_of[X]
                        rbase = rows.start
                        if s_colsplit:
                            for c in range(2):
                                qsl = slice(128 * qt + 64 * c,
                                            128 * qt + 64 * (c + 1))
                                nc.tensor.matmul(
                                    pss[X][64 * c:64 * (c + 1), sl],
                                    qk_sb[rows, qsl],
                                    qk_sb[rows, G + 512 * chk:G + 512 * (chk + 1)],
                                    start=True, stop=False,
                                    tile_position=(rbase, 64 * c),
                                    skip_group_check=True)
                        else:
                            nc.tensor.matmul(
                                pss[X][:, sl],
                                qk_sb[rows, 128 * qt:128 * (qt + 1)],
                                qk_sb[rows, G + 512 * chk:G + 512 * (chk + 1)],
                                start=True, stop=False,
                                skip_group_check=True)
                # mask add via identity matmul (PE), both heads
                for X in "AB":
                    for chk in range(MC):
                        sl = slice(512 * chk, 512 * (chk + 1))
                        nc.tensor.matmul(
                            pss[X][:, sl], ident_sb[:], mbp_sb[:, qt, sl],
                            start=False, stop=(chk == MC - 1),
                            skip_group_check=True)
                for X in "AB":
                    nc.vector.tensor_reduce(
                        negmax[X][:, qt:qt + 1], pss[X][:],
                        axis=mybir.AxisListType.X, op=ALU.max, negate=True)
                    # P = exp(S + mb - max), rowsum
                    P = p_pool.tile([128, G], BF16, tag="p", name=f"P{X}{i}{qt}")
                    nc.scalar.activation(
                        P[:], pss[X][:], AF.Exp,
                        bias=negmax[X][:, qt:qt + 1], scale=1.0,
                        accum_out=rowsum[X][:, qt:qt + 1])
                    nc.vector.reciprocal(rcp[X][:, qt:qt + 1],
                                         rowsum[X][:, qt:qt + 1])
                    if qt % tp_batch == 0:
                        pn2[X] = pn_pool.tile([128, tp_batch, G], BF16,
                                              tag="pn", name=f"pn{X}{i}{qt}")
                    nc.vector.tensor_scalar_mul(pn2[X][:, qt % tp_batch], P[:],
                                                rcp[X][:, qt:qt + 1])
                    if qt % tp_batch == tp_batch - 1:
                        if tp_batch == 2:
                            tp_out = PT[X][:, qt // 2]
                        else:
                            tp_out = PT[X][:, qt // 2, qt % 2]
                        nc.sync.dma_start_transpose(out=tp_out, in_=pn2[X][:])

            # next pair's projections fill the PE bubble while softmax drains
            if i + 1 < len(pairs):
                if hp == NH // 2 - 1:
                    emit_v_batch(b + 1)
                emit_proj(i + 1)

            # --- AV (bf16 K=128, col-paired) + output ---
            o_sb = o_pool.tile([128, QC, 512], FP32, tag="o", name=f"o{i}")
            for qc in range(QC):
                pso = ps_vo.tile([128, 512], FP32, tag="vo", name=f"pso{i}{qc}")
                for mkt in range(MT):
                    st, sp = mkt == 0, mkt == MT - 1
                    nc.tensor.matmul(
                        pso[0:64, :],
                        v_sb[:, b, mkt, 64 * hA:64 * (hA + 1)],
                        PT["A"][:, 2 * qc:2 * qc + 2, :, mkt, :],
                        start=st, stop=sp, skip_group_check=True)
                    nc.tensor.matmul(
                        pso[64:128, :],
                        v_sb[:, b, mkt, 64 * hB:64 * (hB + 1)],
                        PT["B"][:, 2 * qc:2 * qc + 2, :, mkt, :],
                        start=st, stop=sp, skip_group_check=True)
                if qc % 2 == 0:
                    nc.scalar.copy(o_sb[:, qc], pso[:])
                else:
                    nc.vector.tensor_copy(o_sb[:, qc], pso[:])
            nc.gpsimd.dma_start(
                out=out_ext[b, hA].rearrange("d (qc qi) -> d qc qi", qc=QC),
                in_=o_sb[0:64])
            nc.gpsimd.dma_start(
                out=out_ext[b, hB].rearrange("d (qc qi) -> d qc qi", qc=QC),
                in_=o_sb[64:128])

    nc.compile()
    return nc


# ---------------------------------------------------------------------------
# Host-side wrapper: shard over batch across 8 cores, run SPMD, gather.
# ---------------------------------------------------------------------------
import numpy as np
import ml_dtypes

N_CORES = 8
_B_FULL, _NH, _G, _I, _D = 16, 8, 1024, 256, 64
_B_PER_CORE = _B_FULL // N_CORES
_KT = _I // 128

_cached_nc = None


def _get_nc():
    global _cached_nc
    if _cached_nc is None:
        _cached_nc = build_attention(B=_B_PER_CORE, NH=_NH, G=_G, I=_I, D=_D)
    return _cached_nc


def _make_in_maps(h, mask, W_Q, W_K, W_V):
    hT = np.ascontiguousarray(np.transpose(np.asarray(h, np.float32), (0, 2, 1)))
    hTb = hT.astype(ml_dtypes.bfloat16)
    wq = np.ascontiguousarray(np.asarray(W_Q, np.float32) / np.sqrt(np.float32(_D)))
    wk = np.ascontiguousarray(np.asarray(W_K, np.float32))
    # wvp: [KT, 128, NH*D] bf16 packed for the all-heads batched V projection
    wv = np.asarray(W_V, np.float32)                       # [NH, I, D]
    wvp = np.ascontiguousarray(
        wv.transpose(1, 0, 2).reshape(_KT, 128, _NH * _D)
    ).astype(ml_dtypes.bfloat16)
    # mask bias bf16: -1e30 where masked (mask==1), 0 where allowed
    mbp = np.where(np.asarray(mask) != 0, np.float32(-1e30),
                   np.float32(0.0)).astype(ml_dtypes.bfloat16)
    ident = np.eye(128).astype(ml_dtypes.bfloat16)
    return [
        {
            "hT": np.ascontiguousarray(hT[c * _B_PER_CORE:(c + 1) * _B_PER_CORE]),
            "hTb": np.ascontiguousarray(hTb[c * _B_PER_CORE:(c + 1) * _B_PER_CORE]),
            "mbp": mbp,
            "wq": wq,
            "wk": wk,
            "wvp": wvp,
            "ident": ident,
        }
        for c in range(N_CORES)
    ]


def kernel(h, mask, W_Q, W_K, W_V):
    """h [16,1024,256] f32, mask [1024,1024] i32, W_* [8,256,64] f32
    -> [16, 8, 1024, 64] f32"""
    from concourse.bass_utils import run_bass_kernel_spmd

    nc = _get_nc()
    in_maps = _make_in_maps(h, mask, W_Q, W_K, W_V)
    res = run_bass_kernel_spmd(nc, in_maps, core_ids=list(range(N_CORES)))
    outs = [np.asarray(res.results[c]["out"]).reshape(_B_PER_CORE, _NH, _D, _G)
            for c in range(N_CORES)]
    full = np.concatenate(outs, axis=0)              # [16, NH, D, G]
    return np.ascontiguousarray(full.transpose(0, 1, 3, 2)).astype(np.float32)


# revision 11
# speedup vs baseline: 1.4254x; 1.2836x over previous
"""Sparse-attention SPMD kernel (one NeuronCore program), v5.

Per core: B=2 batches x NH=8 heads as 4*B head-pairs (A, B).

Matmul precision strategy: fp32 HW matmuls (LOW_HIGH) cost ~4.8 cyc/col;
instead every contraction runs as 3 bf16 passes (hi*hi + hi*lo + lo*hi
~= 17-bit effective) at ~3 cyc/col:
  - projections: h and W_Q/W_K split hi/lo on the HOST (hTb/hTl, w*h/w*l)
  - S = Q.K^T: Q^T/K^T hi/lo extracted on device: ACT copies psum->hi bf16,
    a negated-identity matmul accumulates -hi into the psum group, DVE
    copies the residual ->lo bf16
  - S MMs: A/B head row-groups run concurrently (row_grp tiling)
  - mask add: identity matmul accumulates -1e30 bias into S psum (both X)
  - DVE: negmax = -max(masked S); ACT: P = exp(S+mb-max) bf16, accum rowsum
  - DVE: reciprocal + Pn = P*rcp (bf16 4x), 2-qt slabs
  - transpose: DMA-xbar per 2-qt slab [128, 2048], sync HWDGE ring only
  - AV: O^T[d,q] bf16 K=128 col-paired A/B
  - V projection batched across all 8 heads (stationary hTb tile, rhs packed
    W_V [128, NH*D] bf16); psum pools alternate to keep PE dense
Output written as O^T [B, NH, 64, G]; host transposes to [B, NH, G, 64].
"""
import sys

sys.path.insert(0, '/opt/trn_rl_repo')
from contextlib import ExitStack

import concourse.bass as bass
import concourse.tile as tile
from concourse import bacc, mybir

FP32 = mybir.dt.float32
BF16 = mybir.dt.bfloat16
AF = mybir.ActivationFunctionType
ALU = mybir.AluOpType


def build_attention(B=2, NH=8, G=1024, I=256, D=64, tp_batch=2):
    assert D == 64 and I % 128 == 0 and G % 512 == 0 and NH % 2 == 0
    KT = I // 128          # contraction k-tiles for projections
    QT = G // 128          # q tiles
    MC = G // 512          # m chunks of 512 (S rhs / psum bank)
    QC = G // 512          # q chunks of 512 (AV psum)
    MT = G // 128          # m tiles
    Q2 = QT // 2           # 2-qt transpose slabs

    nc = bacc.Bacc(None, target_bir_lowering=False, debug=False)
    dram = {}
    for nm, shp, dt in [
        ("hTb", [B, I, G], BF16), ("hTl", [B, I, G], BF16),
        ("mbp", [G, G], BF16),
        ("wqh", [NH, I, D], BF16), ("wql", [NH, I, D], BF16),
        ("wkh", [NH, I, D], BF16), ("wkl", [NH, I, D], BF16),
        ("wvp", [KT, 128, NH * D], BF16),
        ("ident", [128, 128], BF16), ("identn", [128, 128], BF16),
    ]:
        dram[nm] = nc.declare_dram_parameter(nm, shp, dt, isOutput=False)
    out_ext = nc.declare_dram_parameter("out", [B, NH, D, G], FP32,
                                        isOutput=True)

    ctx = ExitStack()
    with ctx:
        tc = ctx.enter_context(tile.TileContext(nc))
        const = ctx.enter_context(tc.tile_pool(name="const", bufs=1))
        vpool = ctx.enter_context(tc.tile_pool(name="vsb", bufs=1))
        qk_pool = ctx.enter_context(tc.tile_pool(name="qk", bufs=2))
        p_pool = ctx.enter_context(tc.tile_pool(name="p", bufs=4))
        pn_pool = ctx.enter_context(tc.tile_pool(name="pn", bufs=3))
        pt_pool = ctx.enter_context(tc.tile_pool(name="pt", bufs=3))
        o_pool = ctx.enter_context(tc.tile_pool(name="o", bufs=2))
        st_pool = ctx.enter_context(tc.tile_pool(name="stats", bufs=2))
        # PSUM budget (8 banks): pss 3x2 + proj 1 + v/o shared 1 = 8
        ps_proj = ctx.enter_context(tc.tile_pool(name="psproj", bufs=1,
                                                 space="PSUM"))
        ps_s = ctx.enter_context(tc.tile_pool(name="pss", bufs=3,
                                              space="PSUM"))
        ps_vo = ctx.enter_context(tc.tile_pool(name="psvo", bufs=1,
                                               space="PSUM"))

        # ---------- setup: load inputs (gpsimd SWDGE ring; sync ring is
        # reserved for the xbar transposes) ----------
        hTb_sb = const.tile([128, B, KT, G], BF16)
        hTl_sb = const.tile([128, B, KT, G], BF16)
        for b in range(B):
            nc.gpsimd.dma_start(
                out=hTb_sb[:, b],
                in_=dram["hTb"][b].rearrange("(kt p) g -> p kt g", p=128))
            nc.gpsimd.dma_start(
                out=hTl_sb[:, b],
                in_=dram["hTl"][b].rearrange("(kt p) g -> p kt g", p=128))

        w_sb = {}
        for nm in ("wqh", "wql", "wkh", "wkl"):
            w_sb[nm] = const.tile([128, NH, KT, D], BF16, name=nm)
            nc.gpsimd.dma_start(
                out=w_sb[nm][:],
                in_=dram[nm].rearrange("h (kt p) d -> p h kt d", p=128))
        wv_sb = const.tile([128, KT, NH * D], BF16)
        nc.gpsimd.dma_start(out=wv_sb[:],
                            in_=dram["wvp"].rearrange("kt p hd -> p kt hd"))

        ident_sb = const.tile([128, 128], BF16)
        identn_sb = const.tile([128, 128], BF16)
        nc.gpsimd.dma_start(out=ident_sb[:], in_=dram["ident"][:])
        nc.gpsimd.dma_start(out=identn_sb[:], in_=dram["identn"][:])

        mbp_sb = const.tile([128, QT, G], BF16)
        nc.gpsimd.dma_start(
            out=mbp_sb[:],
            in_=dram["mbp"].rearrange("(qt p) m -> p qt m", p=128))

        # V for both batches: [128(m), b, mt, NH*D] bf16
        v_sb = vpool.tile([128, B, MT, NH * D], BF16)

        def emit_v_batch(b):
            for mt in range(MT):
                # alternate psum pools so consecutive rounds pipeline
                pl = ps_vo if mt % 2 == 0 else ps_proj
                psv = pl.tile([128, 512], FP32, tag="vo" if mt % 2 == 0
                              else "proj", name=f"psv{b}{mt}")
                for kt in range(KT):
                    nc.tensor.matmul(
                        psv[:], hTb_sb[:, b, kt, 128 * mt:128 * (mt + 1)],
                        wv_sb[:, kt], start=(kt == 0), stop=(kt == KT - 1))
                if mt % 2 == 0:
                    nc.scalar.copy(v_sb[:, b, mt], psv[:])
                else:
                    nc.vector.tensor_copy(v_sb[:, b, mt], psv[:])

        # ---------- main loop over (batch, head-pair), software-pipelined ----
        pairs = [(b, hp) for b in range(B) for hp in range(NH // 2)]
        rows_of = {"A": slice(0, 64), "B": slice(64, 128)}
        state = {}

        def emit_proj(i):
            """Q^T/K^T for pair i: 3-pass bf16, col-paired A/B; psum group is
            then split into hi/lo bf16 tiles (ACT copy, -I matmul, DVE copy).
            """
            b, hp = pairs[i]
            hA, hB = 2 * hp, 2 * hp + 1
            qk_hi = qk_pool.tile([128, 2 * G], BF16, tag="qkh", name=f"qkh{i}")
            qk_lo = qk_pool.tile([128, 2 * G], BF16, tag="qkl", name=f"qkl{i}")
            for W, off in (("q", 0), ("k", G)):
                wh, wl = w_sb["w" + W + "h"], w_sb["w" + W + "l"]
                for chk in range(G // 512):
                    sl = slice(512 * chk, 512 * (chk + 1))
                    psp = ps_proj.tile([128, 512], FP32, tag="proj",
                                       name=f"psp{i}{W}{chk}")
                    first = True
                    for kt in range(KT):
                        for wx, hx in ((wh, hTb_sb), (wh, hTl_sb),
                                       (wl, hTb_sb)):
                            for X, hh in (("A", hA), ("B", hB)):
                                nc.tensor.matmul(
                                    psp[rows_of[X], :], wx[:, hh, kt],
                                    hx[:, b, kt, sl],
                                    start=first, stop=False,
                                    skip_group_check=True)
                            first = False
                    csl = slice(off + 512 * chk, off + 512 * (chk + 1))
                    nc.scalar.copy(qk_hi[:, csl], psp[:])
                    # psp -= hi  (exact residual), then lo = bf16(psp)
                    nc.tensor.matmul(psp[:], identn_sb[:], qk_hi[:, csl],
                                     start=False, stop=True,
                                     skip_group_check=True)
                    nc.vector.tensor_copy(qk_lo[:, csl], psp[:])
            state[i] = (qk_hi, qk_lo)

        emit_proj(0)
        emit_v_batch(0)
        emit_proj(1)
        for i, (b, hp) in enumerate(pairs):
            hA, hB = 2 * hp, 2 * hp + 1
            qk_hi, qk_lo = state.pop(i)

            negmax = {X: st_pool.tile([128, QT], FP32, tag=f"negmax{X}",
                                      name=f"negmax{X}{i}") for X in "AB"}
            rowsum = {X: st_pool.tile([128, QT], FP32, tag=f"rowsum{X}",
                                      name=f"rowsum{X}{i}") for X in "AB"}
            rcp = {X: st_pool.tile([128, QT], FP32, tag=f"rcp{X}",
                                   name=f"rcp{X}{i}") for X in "AB"}
            # PT layout: [m_in(128), qt2, qt_lo(2), mt(8), q_in(128)]
            PT = {X: pt_pool.tile([128, Q2, 2, MT, 128], BF16, tag="pt",
                                  name=f"PT{X}{i}") for X in "AB"}
            pn2 = {}
            for qt in range(QT):
                pss = {X: ps_s.tile([128, G], FP32, tag="s",
                                    name=f"pss{X}{i}{qt}") for X in "AB"}
                # S: 3 bf16 passes per (qt, chunk, X); A/B row groups overlap
                qsl = slice(128 * qt, 128 * (qt + 1))
                for chk in range(MC):
                    sl = slice(512 * chk, 512 * (chk + 1))
                    ksl = slice(G + 512 * chk, G + 512 * (chk + 1))
                    for lh, rh, st in ((qk_hi, qk_hi, True),
                                       (qk_hi, qk_lo, False),
                                       (qk_lo, qk_hi, False)):
                        for X in "AB":
                            rows = rows_of[X]
                            nc.tensor.matmul(
                                pss[X][:, sl], lh[rows, qsl], rh[rows, ksl],
                                start=st, stop=False, skip_group_check=True)
                # mask add via identity matmul (PE), both heads
                for X in "AB":
                    for chk in range(MC):
                        sl = slice(512 * chk, 512 * (chk + 1))
                        nc.tensor.matmul(
                            pss[X][:, sl], ident_sb[:], mbp_sb[:, qt, sl],
                            start=False, stop=(chk == MC - 1),
                            skip_group_check=True)
                for X in "AB":
                    nc.vector.tensor_reduce(
                        negmax[X][:, qt:qt + 1], pss[X][:],
                        axis=mybir.AxisListType.X, op=ALU.max, negate=True)
                    P = p_pool.tile([128, G], BF16, tag="p", name=f"P{X}{i}{qt}")
                    nc.scalar.activation(
                        P[:], pss[X][:], AF.Exp,
                        bias=negmax[X][:, qt:qt + 1], scale=1.0,
                        accum_out=rowsum[X][:, qt:qt + 1])
                    nc.vector.reciprocal(rcp[X][:, qt:qt + 1],
                                         rowsum[X][:, qt:qt + 1])
                    if qt % tp_batch == 0:
                        pn2[X] = pn_pool.tile([128, tp_batch, G], BF16,
                                              tag="pn", name=f"pn{X}{i}{qt}")
                    nc.vector.tensor_scalar_mul(pn2[X][:, qt % tp_batch], P[:],
                                                rcp[X][:, qt:qt + 1])
                    if qt % tp_batch == tp_batch - 1:
                        if tp_batch == 2:
                            tp_out = PT[X][:, qt // 2]
                        else:
                            tp_out = PT[X][:, qt // 2, qt % 2]
                        nc.sync.dma_start_transpose(out=tp_out, in_=pn2[X][:])

            # next pair's projections fill the PE bubble while softmax drains
            if i + 2 < len(pairs):
                emit_proj(i + 2)

            # --- AV (bf16 K=128, col-paired) + output ---
            o_sb = o_pool.tile([128, QC, 512], FP32, tag="o", name=f"o{i}")
            for qc in range(QC):
                pl = ps_vo if qc % 2 == 0 else ps_proj
                pso = pl.tile([128, 512], FP32, tag="vo" if qc % 2 == 0
                              else "proj", name=f"pso{i}{qc}")
                for mkt in range(MT):
                    st, sp = mkt == 0, mkt == MT - 1
                    nc.tensor.matmul(
                        pso[0:64, :],
                        v_sb[:, b, mkt, 64 * hA:64 * (hA + 1)],
                        PT["A"][:, 2 * qc:2 * qc + 2, :, mkt, :],
                        start=st, stop=sp, skip_group_check=True)
                    nc.tensor.matmul(
                        pso[64:128, :],
                        v_sb[:, b, mkt, 64 * hB:64 * (hB + 1)],
                        PT["B"][:, 2 * qc:2 * qc + 2, :, mkt, :],
                        start=st, stop=sp, skip_group_check=True)
                if qc % 2 == 0:
                    nc.scalar.copy(o_sb[:, qc], pso[:])
                else:
                    nc.vector.tensor_copy(o_sb[:, qc], pso[:])
            nc.gpsimd.dma_start(
                out=out_ext[b, hA].rearrange("d (qc qi) -> d qc qi", qc=QC),
                in_=o_sb[0:64])
            nc.gpsimd.dma_start(
                out=out_ext[b, hB].rearrange("d (qc qi) -> d qc qi", qc=QC),
                in_=o_sb[64:128])
            if hp == NH // 2 - 1 and b + 1 < B:
                emit_v_batch(b + 1)

    nc.compile()
    return nc


# ---------------------------------------------------------------------------
# Host-side wrapper: shard over batch across 8 cores, run SPMD, gather.
# ---------------------------------------------------------------------------
import numpy as np
import ml_dtypes

N_CORES = 8
_B_FULL, _NH, _G, _I, _D = 16, 8, 1024, 256, 64
_B_PER_CORE = _B_FULL // N_CORES
_KT = _I // 128

_cached_nc = None


def _get_nc():
    global _cached_nc
    if _cached_nc is None:
        _cached_nc = build_attention(B=_B_PER_CORE, NH=_NH, G=_G, I=_I, D=_D)
    return _cached_nc


def _split_bf16(x):
    hi = x.astype(ml_dtypes.bfloat16)
    lo = (x - hi.astype(np.float32)).astype(ml_dtypes.bfloat16)
    return hi, lo


def _make_in_maps(h, mask, W_Q, W_K, W_V):
    hT = np.ascontiguousarray(np.transpose(np.asarray(h, np.float32), (0, 2, 1)))
    hTb, hTl = _split_bf16(hT)
    wq = np.asarray(W_Q, np.float32) / np.sqrt(np.float32(_D))
    wk = np.asarray(W_K, np.float32)
    wqh, wql = _split_bf16(wq)
    wkh, wkl = _split_bf16(wk)
    wv = np.asarray(W_V, np.float32)                       # [NH, I, D]
    wvp = np.ascontiguousarray(
        wv.transpose(1, 0, 2).reshape(_KT, 128, _NH * _D)
    ).astype(ml_dtypes.bfloat16)
    mbp = np.where(np.asarray(mask) != 0, np.float32(-1e30),
                   np.float32(0.0)).astype(ml_dtypes.bfloat16)
    ident = np.eye(128).astype(ml_dtypes.bfloat16)
    identn = (-np.eye(128)).astype(ml_dtypes.bfloat16)
    return [
        {
            "hTb": np.ascontiguousarray(hTb[c * _B_PER_CORE:(c + 1) * _B_PER_CORE]),
            "hTl": np.ascontiguousarray(hTl[c * _B_PER_CORE:(c + 1) * _B_PER_CORE]),
            "mbp": mbp,
            "wqh": np.ascontiguousarray(wqh), "wql": np.ascontiguousarray(wql),
            "wkh": np.ascontiguousarray(wkh), "wkl": np.ascontiguousarray(wkl),
            "wvp": wvp,
            "ident": ident, "identn": identn,
        }
        for c in range(N_CORES)
    ]


def kernel(h, mask, W_Q, W_K, W_V):
    """h [16,1024,256] f32, mask [1024,1024] i32, W_* [8,256,64] f32
    -> [16, 8, 1024, 64] f32"""
    from concourse.bass_utils import run_bass_kernel_spmd

    nc = _get_nc()
    in_maps = _make_in_maps(h, mask, W_Q, W_K, W_V)
    res = run_bass_kernel_spmd(nc, in_maps, core_ids=list(range(N_CORES)))
    outs = [np.asarray(res.results[c]["out"]).reshape(_B_PER_CORE, _NH, _D, _G)
            for c in range(N_CORES)]
    full = np.concatenate(outs, axis=0)              # [16, NH, D, G]
    return np.ascontiguousarray(full.transpose(0, 1, 3, 2)).astype(np.float32)
